# revision 23
# baseline (speedup 1.0000x reference)
"""Boundary-map kernel for Trainium2 (Bass, raw engine streams), 8-core SPMD.

Math: a pixel is an edge pixel iff its radius-2 Euclidean disk (clipped to the
zero-padded array) contains both a 1 and a 0 of some class's one-hot map.
Equivalently (disk is 4-connected): there exists a 4-adjacent pair of pixels
inside the disk with different labels, OR the disk is uniform-nonzero and
touches the pad ring.  With label maps zero-padded by 2, let
    DH(i,j) = [x(i,j) != x(i,j+1)],   DV(i,j) = [x(i+1,j) != x(i,j)]
and dilate each by the set of in-disk pair positions:
    SH = {(0,-2),(0,-1),(0,0),(0,1),(+-1,-1),(+-1,0)}
    SV = {(-2,0),(-1,0),(0,0),(1,0),(-1,+-1),(0,+-1)}
    edge = (sum_{s in SH} DH(p+s) + sum_{s in SV} DV(p+s)) > 0
The zero pad makes the pad-adjacent DV terms fire exactly when the reference's
border term (disk touches pad AND has a nonzero) fires, so no explicit border
handling is needed (verified exhaustively against the reference in numpy).

Layout: ONE [128 partitions x 4620 free] bf16 tile per core, free dim packing
three row-band segments side by side (each with its own 2-col halos):
    seg3 = strip rows       x 516 cols   -> last 32 rows, 512-col slice
                                            (36 partitions only, K=36 matmuls)
    seg1 = rows base+p      x 2052 cols  -> output rows 0..123
    seg2 = rows base+124+p  x 2052 cols  -> output rows 124..247
      (8 cores x 248 rows = 1984 rows; strips cover the last 64 rows)
The host duplicates the NEXT row of every segment in cols [4620, 9240), so
the vertical not_equal DV is a plain free-dim DVE op; all elementwise ops run
on DVE in 2x perf mode.  Row-tap dilation is band-matrix matmuls on the
TensorEngine (36 passes, weight-major per segment); thresholds are ACT Sign /
DVE is_gt ops writing int8, split across both engines to shorten the tail.

Pipeline: inputs stream per segment over BOTH HWDGE queues (sync + scalar
issue) — the tiny strip segment first, so its DVE/PE work fills the window
while seg1's data is still in flight, and its PSUM bank (shared with seg2's
last chunk) is retired long before seg2 reaches it; each segment's DVE chain
overlaps the previous segment's matmuls, and the last DVE op (seg2's H4p) is
split into 512-col pieces so the final PE pass streams right behind it.
Synchronization is fully manual (~13 semaphores), no TileContext — no
per-instruction sem traffic and no tail drain.
"""

import numpy as np
import ml_dtypes

import concourse.bass as bass
import concourse.bacc as bacc
import concourse.mybir as mybir
from concourse import bass_utils

BF16 = mybir.dt.bfloat16
F32 = mybir.dt.float32
I8 = mybir.dt.int8
OP = mybir.AluOpType
AF = mybir.ActivationFunctionType

B, H, W = 2, 1024, 2048
RPC = 248            # rows per core from full-width segments (2 x 124)
SR, SC = 32, 512     # strip rows / cols per core
CT = 4620            # 516 + 2052 + 2052 free cols (current rows)
NCORES = 8
CHUNK = 512

PROFILE = False
LAST_EXEC_NS = None
LAST_RESULTS = None

WNAMES = ("w_11", "w_i", "w_v4", "w_v2")
# segment col starts/widths in the packed tile: strip, band1, band2
SEG_S = (0, 516, 2568)
SEG_L = (516, 2052, 2052)
# matmul chunk starts per segment (j0; output col = j0 - seg_start - 2)
SEG_J0 = ([2], [516 + 2 + k * CHUNK for k in range(4)],
          [2568 + 2 + k * CHUNK for k in range(4)])
# e1 / y2 col base per segment
SEG_E = (4096, 0, 2048)


def _band(taps, P=128):
    w = np.zeros((P, P), np.float32)  # [k, m]: out row m sums w[k,m]*src[k]
    for m in range(P):
        for t, v in taps:
            k = m + t
            if 0 <= k < P:
                w[k, m] += v
    return w.astype(ml_dtypes.bfloat16)


def make_weights():
    wd = {
        "w_11": _band([(-1, 1.0), (1, 1.0)]),                       # taps m-1, m+1
        "w_i": _band([(0, 1.0)]),                                   # identity
        "w_v4": _band([(-2, 1.0), (-1, 1.0), (0, 1.0), (1, 1.0)]),  # taps m-2..m+1
        "w_v2": _band([(-1, 1.0), (0, 1.0)]),                       # taps m-1, m
    }
    return np.concatenate([wd[k] for k in WNAMES], axis=1)


def build_nc():
    nc = bacc.Bacc("TRN2", target_bir_lowering=False, debug=False)
    x = nc.dram_tensor("x", [128, 2 * 4104], BF16, kind="ExternalInput").ap()
    xs = nc.dram_tensor("xs", [36, 2 * 516], BF16, kind="ExternalInput").ap()
    wcat = nc.dram_tensor("wcat", [128, 128 * len(WNAMES)], BF16,
                          kind="ExternalInput").ap()
    y2 = nc.dram_tensor("y2", [124, 4608], I8, kind="ExternalOutput").ap()

    xi = nc.alloc_sbuf_tensor("xi", [128, 2 * CT], BF16)
    wt_t = nc.alloc_sbuf_tensor("wt", [128, 128 * len(WNAMES)], BF16)
    DH = nc.alloc_sbuf_tensor("DH", [128, CT], BF16)
    DV = nc.alloc_sbuf_tensor("DV", [128, CT], BF16)
    H2 = nc.alloc_sbuf_tensor("H2", [128, CT], BF16)
    H4p = nc.alloc_sbuf_tensor("H4p", [128, CT], BF16)
    DVHp = nc.alloc_sbuf_tensor("DVHp", [128, CT], BF16)
    e1 = nc.alloc_sbuf_tensor("e1", [128, 4608], I8)
    pA = nc.alloc_psum_tensor("pA", [128, 2048], F32)
    pB = nc.alloc_psum_tensor("pB", [128, 2048], F32)

    wt = {k: wt_t[:, 128 * i:128 * (i + 1)] for i, k in enumerate(WNAMES)}

    wsem = nc.alloc_semaphore("wsem")
    csems = [nc.alloc_semaphore(f"csem{i}") for i in range(3)]
    nsems = [nc.alloc_semaphore(f"nsem{i}") for i in range(3)]
    vsem = nc.alloc_semaphore("vsem")
    psem = nc.alloc_semaphore("psem")
    a1sem = nc.alloc_semaphore("a1sem")
    a2sem = nc.alloc_semaphore("a2sem")
    a3sem = nc.alloc_semaphore("a3sem")
    osem = nc.alloc_semaphore("osem")

    C = CT

    def psum_dst(seg, k):
        # strip shares pB's LAST bank (seg2 chunk 3 runs long after the
        # strip's threshold retired it)
        if seg == 0:
            return pB[:, 3 * CHUNK:4 * CHUNK]
        return (pB if seg == 2 else pA)[:, k * CHUNK:(k + 1) * CHUNK]

    # PE pass order matches DVE production order: H2, DV, DVHp, H4p
    passes = [("w_11", H2, 0, 2), ("w_v4", DV, 0, 3),
              ("w_v2", DVHp, -1, 4), ("w_i", H4p, -1, 5)]
    # psem index for (seg, chunk): strip = 1, seg1 c0-3 = 2-5, seg2 = 6-9
    def psem_idx(seg, k):
        return {0: 1, 1: 2 + k, 2: 6 + k}[seg]

    with nc.Block() as blk:

        @blk.sync
        def _(eng):
            eng.dma_start(xi[0:36, 0:516], xs[:, 0:516]).then_inc(csems[0], 16)
            eng.dma_start(wt_t[:, :], wcat).then_inc(wsem, 16)
            eng.dma_start(xi[:, 516:2568], x[:, 0:2052]).then_inc(csems[1], 16)
            eng.dma_start(xi[:, 2568:4620], x[:, 2052:4104]).then_inc(csems[2], 16)
            eng.wait_ge(a3sem, 1)
            eng.dma_start(y2[:, 4096:4608], e1[2:126, 4096:4608]).then_inc(osem, 16)
            eng.wait_ge(a1sem, 4)
            eng.dma_start(y2[:, 0:2048], e1[2:126, 0:2048]).then_inc(osem, 16)
            eng.wait_ge(a2sem, 4)
            eng.dma_start(y2[:, 2048:4096], e1[2:126, 2048:4096]).then_inc(osem, 16)
            eng.wait_ge(osem, 48)

        @blk.vector
        def _(eng):
            # per-segment elementwise chains
            vsb = {0: 0, 1: 5, 2: 10}
            for seg in (0, 1, 2):
                S, L = SEG_S[seg], SEG_L[seg]
                E = S + L
                vs = vsb[seg]
                P = 36 if seg == 0 else 128
                eng.wait_ge(csems[seg], 16)
                # DH(j) = [x(j) != x(j+1)], valid [S, E-1)
                eng.tensor_tensor(out=DH[0:P, S:E - 1], in0=xi[0:P, S:E - 1],
                                  in1=xi[0:P, S + 1:E],
                                  op=OP.not_equal).then_inc(vsem, 1)
                # H2(j) = DH(j-1) + DH(j), valid [S+1, E-1)
                eng.wait_ge(vsem, vs + 1)
                eng.tensor_tensor(out=H2[0:P, S + 1:E - 1], in0=DH[0:P, S:E - 2],
                                  in1=DH[0:P, S + 1:E - 1],
                                  op=OP.add).then_inc(vsem, 1)
                # DV(j) = [x(j) != next(j)], valid [S, E)
                eng.wait_ge(nsems[seg], 16)
                eng.tensor_tensor(out=DV[0:P, S:E], in0=xi[0:P, S:E],
                                  in1=xi[0:P, C + S:C + E],
                                  op=OP.not_equal).then_inc(vsem, 1)
                # DVHp(j) = DV(j) + DV(j+2), valid [S, E-2)
                eng.wait_ge(vsem, vs + 3)
                eng.tensor_tensor(out=DVHp[0:P, S:E - 2], in0=DV[0:P, S:E - 2],
                                  in1=DV[0:P, S + 2:E],
                                  op=OP.add).then_inc(vsem, 1)
                # H4p(j) = H2(j) + H2(j+2), valid [S+1, E-3)
                eng.wait_ge(vsem, vs + 2)
                if seg < 2:
                    eng.tensor_tensor(out=H4p[0:P, S + 1:E - 3],
                                      in0=H2[0:P, S + 1:E - 3],
                                      in1=H2[0:P, S + 3:E - 1],
                                      op=OP.add).then_inc(vsem, 1)
                else:
                    # chunk pieces so the final PE pass streams right behind
                    for k in range(4):
                        a = S + 1 + k * CHUNK
                        eng.tensor_tensor(out=H4p[:, a:a + CHUNK],
                                          in0=H2[:, a:a + CHUNK],
                                          in1=H2[:, a + 2:a + CHUNK + 2],
                                          op=OP.add).then_inc(vsem, 1)
            # seg2 chunk 2/3 thresholds (ACT handles the rest)
            for k in (2, 3):
                eng.wait_ge(psem, psem_idx(2, k))
                eng.tensor_scalar(out=e1[:, SEG_E[2] + k * CHUNK:
                                         SEG_E[2] + (k + 1) * CHUNK],
                                  in0=psum_dst(2, k), scalar1=0.0, scalar2=None,
                                  op0=OP.is_gt).then_inc(a2sem, 1)

        @blk.tensor
        def _(eng):
            eng.wait_ge(wsem, 16)
            for seg in (0, 1, 2):
                vbase = {0: 0, 1: 5, 2: 10}[seg]
                for wi, (wname, rhs, doff, need) in enumerate(passes):
                    if not (seg == 2 and wi == 3):
                        eng.wait_ge(vsem, vbase + need)
                    for k, j0 in enumerate(SEG_J0[seg]):
                        if seg == 2 and wi == 3:
                            eng.wait_ge(vsem, 15 + k)
                        if seg == 2 and wi == 0 and k == 3:
                            # strip's threshold must retire pB bank 3 first
                            eng.wait_ge(a3sem, 1)
                        j = j0 + doff
                        st = (wi == 0)
                        K = 36 if seg == 0 else 128
                        mm = eng.matmul(out=psum_dst(seg, k),
                                        lhsT=wt[wname][0:K, :],
                                        rhs=rhs[0:K, j:j + CHUNK],
                                        start=st, stop=(wi == 3),
                                        skip_group_check=True)
                        if wi == 3:
                            mm.then_inc(psem, 1)

        @blk.scalar
        def _(eng):
            # second input halves (NEXT rows) on the ACT HWDGE queue
            eng.dma_start(xi[0:36, C:C + 516], xs[:, 516:1032]).then_inc(nsems[0], 16)
            eng.dma_start(xi[:, C + 516:C + 2568],
                          x[:, 4104:6156]).then_inc(nsems[1], 16)
            eng.dma_start(xi[:, C + 2568:C + 4620],
                          x[:, 6156:8208]).then_inc(nsems[2], 16)
            # thresholds: strip, then seg1 c0-3, then seg2 c0-1
            eng.wait_ge(psem, psem_idx(0, 0))
            eng.activation(out=e1[:, 4096:4608], in_=psum_dst(0, 0),
                           func=AF.Sign).then_inc(a3sem, 1)
            for k in range(4):
                eng.wait_ge(psem, psem_idx(1, k))
                eng.activation(out=e1[:, k * CHUNK:(k + 1) * CHUNK],
                               in_=psum_dst(1, k),
                               func=AF.Sign).then_inc(a1sem, 1)
            for k in (0, 1):
                eng.wait_ge(psem, psem_idx(2, k))
                eng.activation(out=e1[:, SEG_E[2] + k * CHUNK:
                                      SEG_E[2] + (k + 1) * CHUNK],
                               in_=psum_dst(2, k),
                               func=AF.Sign).then_inc(a2sem, 1)

    nc.compile()
    return nc


def make_in_maps(gtmasks):
    lab = np.asarray(gtmasks)[:, 0].astype(ml_dtypes.bfloat16)  # labels 0..19
    wcat = make_weights()
    # one extra bottom pad row so the strip's (unused) last DV row has data
    padded = [np.pad(lab[b], ((2, 3), (2, 2))) for b in range(B)]
    in_maps = []
    rows128 = np.arange(128)
    for c in range(NCORES):
        b, q = divmod(c, B * 2)  # 4 cores per batch
        xf = padded[b]
        base = RPC * q

        def seg_block(shift):
            s1 = xf[base + shift + rows128, :]                    # [128, 2052]
            s2 = xf[base + 124 + shift + rows128, :]              # [128, 2052]
            return np.concatenate([s1, s2], axis=1)

        def strip_block(shift):
            return xf[H - SR + shift: H - SR + shift + 36,
                      SC * q: SC * q + SEG_L[0]]

        xfull = np.concatenate([seg_block(0), seg_block(1)], axis=1)
        xstrip = np.concatenate([strip_block(0), strip_block(1)], axis=1)
        in_maps.append({"x": np.ascontiguousarray(xfull),
                        "xs": np.ascontiguousarray(xstrip), "wcat": wcat})
    return in_maps


def assemble(results):
    out = np.zeros((B, 1, H, W), np.int32)
    for c in range(NCORES):
        b, q = divmod(c, B * 2)
        y2 = results[c]["y2"]
        out[b, 0, RPC * q: RPC * q + 124, :] = y2[:, 0:2048]
        out[b, 0, RPC * q + 124: RPC * q + 248, :] = y2[:, 2048:4096]
        out[b, 0, H - SR:, SC * q: SC * q + SC] = y2[0:SR, 4096:4608]
    return out


def kernel(gtmasks):
    global LAST_EXEC_NS, LAST_RESULTS
    in_maps = make_in_maps(gtmasks)
    nc = build_nc()
    res = bass_utils.run_bass_kernel_spmd(
        nc, in_maps, core_ids=list(range(NCORES)), trace=PROFILE)
    LAST_EXEC_NS = res.exec_time_ns
    LAST_RESULTS = res
    return assemble(res.results)


# revision 24
# speedup vs baseline: 1.0128x; 1.0128x over previous
"""Boundary-map kernel for Trainium2 (Bass, raw engine streams), 8-core SPMD.

Math: a pixel is an edge pixel iff its radius-2 Euclidean disk (clipped to the
zero-padded array) contains both a 1 and a 0 of some class's one-hot map.
Equivalently (disk is 4-connected): there exists a 4-adjacent pair of pixels
inside the disk with different labels, OR the disk is uniform-nonzero and
touches the pad ring.  With label maps zero-padded by 2, let
    DH(i,j) = [x(i,j) != x(i,j+1)],   DV(i,j) = [x(i+1,j) != x(i,j)]
and dilate each by the set of in-disk pair positions:
    SH = {(0,-2),(0,-1),(0,0),(0,1),(+-1,-1),(+-1,0)}
    SV = {(-2,0),(-1,0),(0,0),(1,0),(-1,+-1),(0,+-1)}
    edge = (sum_{s in SH} DH(p+s) + sum_{s in SV} DV(p+s)) > 0
The zero pad makes the pad-adjacent DV terms fire exactly when the reference's
border term (disk touches pad AND has a nonzero) fires, so no explicit border
handling is needed (verified exhaustively against the reference in numpy).

Layout: ONE [128 partitions x 4620 free] bf16 tile per core, free dim packing
three row-band segments side by side (each with its own 2-col halos):
    seg3 = strip rows       x 516 cols   -> last 32 rows, 512-col slice
                                            (36 partitions only, K=36 matmuls)
    seg1 = rows base+p      x 2052 cols  -> output rows 0..123
    seg2 = rows base+124+p  x 2052 cols  -> output rows 124..247
      (8 cores x 248 rows = 1984 rows; strips cover the last 64 rows)
The host duplicates the NEXT row of every segment in cols [4620, 9240), so
the vertical not_equal DV is a plain free-dim DVE op; all elementwise ops run
on DVE in 2x perf mode.  Row-tap dilation is band-matrix matmuls on the
TensorEngine (36 passes, weight-major per segment); thresholds are ACT Sign /
DVE is_gt ops writing int8, split across both engines to shorten the tail.

Pipeline: inputs stream per segment over BOTH HWDGE queues (sync + scalar
issue) — the tiny strip segment first, so its DVE/PE work fills the window
while seg1's data is still in flight, and its PSUM bank (shared with seg2's
last chunk) is retired long before seg2 reaches it; each segment's DVE chain
overlaps the previous segment's matmuls, and the last DVE op (seg2's H4p) is
split into 512-col pieces so the final PE pass streams right behind it.
Synchronization is fully manual (~13 semaphores), no TileContext — no
per-instruction sem traffic and no tail drain.
"""

import numpy as np
import ml_dtypes

import concourse.bass as bass
import concourse.bacc as bacc
import concourse.mybir as mybir
from concourse import bass_utils

BF16 = mybir.dt.bfloat16
F32 = mybir.dt.float32
I8 = mybir.dt.int8
OP = mybir.AluOpType
AF = mybir.ActivationFunctionType

B, H, W = 2, 1024, 2048
RPC = 248            # rows per core from full-width segments (2 x 124)
SR, SC = 32, 512     # strip rows / cols per core
CT = 4620            # 516 + 2052 + 2052 free cols (current rows)
NCORES = 8
CHUNK = 512

PROFILE = False
LAST_EXEC_NS = None
LAST_RESULTS = None

WNAMES = ("w_11", "w_i", "w_v4", "w_v2")
# segment col starts/widths in the packed tile: strip, band1, band2
SEG_S = (0, 516, 2568)
SEG_L = (516, 2052, 2052)
# matmul chunk starts per segment (j0; output col = j0 - seg_start - 2)
SEG_J0 = ([2], [516 + 2 + k * CHUNK for k in range(4)],
          [2568 + 2 + k * CHUNK for k in range(4)])
# e1 / y2 col base per segment
SEG_E = (4096, 0, 2048)


def _band(taps, P=128):
    w = np.zeros((P, P), np.float32)  # [k, m]: out row m sums w[k,m]*src[k]
    for m in range(P):
        for t, v in taps:
            k = m + t
            if 0 <= k < P:
                w[k, m] += v
    return w.astype(ml_dtypes.bfloat16)


def make_weights():
    wd = {
        "w_11": _band([(-1, 1.0), (1, 1.0)]),                       # taps m-1, m+1
        "w_i": _band([(0, 1.0)]),                                   # identity
        "w_v4": _band([(-2, 1.0), (-1, 1.0), (0, 1.0), (1, 1.0)]),  # taps m-2..m+1
        "w_v2": _band([(-1, 1.0), (0, 1.0)]),                       # taps m-1, m
    }
    return np.concatenate([wd[k] for k in WNAMES], axis=1)


def build_nc():
    nc = bacc.Bacc("TRN2", target_bir_lowering=False, debug=False)
    x = nc.dram_tensor("x", [128, 2 * 4104], BF16, kind="ExternalInput").ap()
    xs = nc.dram_tensor("xs", [36, 2 * 516], BF16, kind="ExternalInput").ap()
    wcat = nc.dram_tensor("wcat", [128, 128 * len(WNAMES)], BF16,
                          kind="ExternalInput").ap()
    y2 = nc.dram_tensor("y2", [124, 4608], I8, kind="ExternalOutput").ap()

    xi = nc.alloc_sbuf_tensor("xi", [128, 2 * CT], BF16)
    wt_t = nc.alloc_sbuf_tensor("wt", [128, 128 * len(WNAMES)], BF16)
    DH = nc.alloc_sbuf_tensor("DH", [128, CT], BF16)
    DV = nc.alloc_sbuf_tensor("DV", [128, CT], BF16)
    H2 = nc.alloc_sbuf_tensor("H2", [128, CT], BF16)
    H4p = nc.alloc_sbuf_tensor("H4p", [128, CT], BF16)
    DVHp = nc.alloc_sbuf_tensor("DVHp", [128, CT], BF16)
    e1 = nc.alloc_sbuf_tensor("e1", [128, 4608], I8)
    pA = nc.alloc_psum_tensor("pA", [128, 2048], F32)
    pB = nc.alloc_psum_tensor("pB", [128, 2048], F32)

    wt = {k: wt_t[:, 128 * i:128 * (i + 1)] for i, k in enumerate(WNAMES)}

    wsem = nc.alloc_semaphore("wsem")
    csems = [nc.alloc_semaphore(f"csem{i}") for i in range(3)]
    nsems = [nc.alloc_semaphore(f"nsem{i}") for i in range(3)]
    vsem = nc.alloc_semaphore("vsem")
    psem = nc.alloc_semaphore("psem")
    a1sem = nc.alloc_semaphore("a1sem")
    a2sem = nc.alloc_semaphore("a2sem")
    a3sem = nc.alloc_semaphore("a3sem")
    osem = nc.alloc_semaphore("osem")

    C = CT

    def psum_dst(seg, k):
        # strip shares pB's LAST bank (seg2 chunk 3 runs long after the
        # strip's threshold retired it)
        if seg == 0:
            return pB[:, 3 * CHUNK:4 * CHUNK]
        return (pB if seg == 2 else pA)[:, k * CHUNK:(k + 1) * CHUNK]

    # PE pass order matches DVE production order: H2, DV, DVHp, H4p
    passes = [("w_11", H2, 0, 2), ("w_v4", DV, 0, 3),
              ("w_v2", DVHp, -1, 4), ("w_i", H4p, -1, 5)]
    # psem index for (seg, chunk): strip = 1, seg1 c0-3 = 2-5, seg2 = 6-9
    def psem_idx(seg, k):
        return {0: 1, 1: 2 + k, 2: 6 + k}[seg]

    with nc.Block() as blk:

        @blk.sync
        def _(eng):
            eng.dma_start(xi[0:36, 0:516], xs[:, 0:516]).then_inc(csems[0], 16)
            eng.dma_start(xi[:, 516:2568], x[:, 0:2052]).then_inc(csems[1], 16)
            eng.dma_start(xi[:, 2568:4620], x[:, 2052:4104]).then_inc(csems[2], 16)
            eng.wait_ge(a3sem, 1)
            eng.dma_start(y2[:, 4096:4608], e1[2:126, 4096:4608]).then_inc(osem, 16)
            eng.wait_ge(a1sem, 4)
            eng.dma_start(y2[:, 0:2048], e1[2:126, 0:2048]).then_inc(osem, 16)
            eng.wait_ge(a2sem, 4)
            eng.dma_start(y2[:, 2048:4096], e1[2:126, 2048:4096]).then_inc(osem, 16)
            eng.wait_ge(osem, 48)

        @blk.vector
        def _(eng):
            # per-segment elementwise chains
            vsb = {0: 0, 1: 5, 2: 10}
            for seg in (0, 1, 2):
                S, L = SEG_S[seg], SEG_L[seg]
                E = S + L
                vs = vsb[seg]
                P = 36 if seg == 0 else 128
                eng.wait_ge(csems[seg], 16)
                # DH(j) = [x(j) != x(j+1)], valid [S, E-1)
                eng.tensor_tensor(out=DH[0:P, S:E - 1], in0=xi[0:P, S:E - 1],
                                  in1=xi[0:P, S + 1:E],
                                  op=OP.not_equal).then_inc(vsem, 1)
                # H2(j) = DH(j-1) + DH(j), valid [S+1, E-1)
                eng.wait_ge(vsem, vs + 1)
                eng.tensor_tensor(out=H2[0:P, S + 1:E - 1], in0=DH[0:P, S:E - 2],
                                  in1=DH[0:P, S + 1:E - 1],
                                  op=OP.add).then_inc(vsem, 1)
                # DV(j) = [x(j) != next(j)], valid [S, E)
                eng.wait_ge(nsems[seg], 16)
                eng.tensor_tensor(out=DV[0:P, S:E], in0=xi[0:P, S:E],
                                  in1=xi[0:P, C + S:C + E],
                                  op=OP.not_equal).then_inc(vsem, 1)
                # DVHp(j) = DV(j) + DV(j+2), valid [S, E-2)
                eng.wait_ge(vsem, vs + 3)
                eng.tensor_tensor(out=DVHp[0:P, S:E - 2], in0=DV[0:P, S:E - 2],
                                  in1=DV[0:P, S + 2:E],
                                  op=OP.add).then_inc(vsem, 1)
                # H4p(j) = H2(j) + H2(j+2), valid [S+1, E-3)
                eng.wait_ge(vsem, vs + 2)
                if seg < 2:
                    eng.tensor_tensor(out=H4p[0:P, S + 1:E - 3],
                                      in0=H2[0:P, S + 1:E - 3],
                                      in1=H2[0:P, S + 3:E - 1],
                                      op=OP.add).then_inc(vsem, 1)
                else:
                    # chunk pieces so the final PE pass streams right behind
                    for k in range(4):
                        a = S + 1 + k * CHUNK
                        eng.tensor_tensor(out=H4p[:, a:a + CHUNK],
                                          in0=H2[:, a:a + CHUNK],
                                          in1=H2[:, a + 2:a + CHUNK + 2],
                                          op=OP.add).then_inc(vsem, 1)
            # seg2 chunk 2/3 thresholds (ACT handles the rest)
            for k in (2, 3):
                eng.wait_ge(psem, psem_idx(2, k))
                eng.tensor_scalar(out=e1[:, SEG_E[2] + k * CHUNK:
                                         SEG_E[2] + (k + 1) * CHUNK],
                                  in0=psum_dst(2, k), scalar1=0.0, scalar2=None,
                                  op0=OP.is_gt).then_inc(a2sem, 1)

        @blk.tensor
        def _(eng):
            eng.wait_ge(wsem, 16)
            for seg in (0, 1, 2):
                vbase = {0: 0, 1: 5, 2: 10}[seg]
                for wi, (wname, rhs, doff, need) in enumerate(passes):
                    if not (seg == 2 and wi == 3):
                        eng.wait_ge(vsem, vbase + need)
                    for k, j0 in enumerate(SEG_J0[seg]):
                        if seg == 2 and wi == 3:
                            eng.wait_ge(vsem, 15 + k)
                        if seg == 2 and wi == 0 and k == 3:
                            # strip's threshold must retire pB bank 3 first
                            eng.wait_ge(a3sem, 1)
                        j = j0 + doff
                        st = (wi == 0)
                        K = 36 if seg == 0 else 128
                        mm = eng.matmul(out=psum_dst(seg, k),
                                        lhsT=wt[wname][0:K, :],
                                        rhs=rhs[0:K, j:j + CHUNK],
                                        start=st, stop=(wi == 3),
                                        skip_group_check=True)
                        if wi == 3:
                            mm.then_inc(psem, 1)

        @blk.scalar
        def _(eng):
            # second input halves (NEXT rows) on the ACT HWDGE queue
            eng.dma_start(xi[0:36, C:C + 516], xs[:, 516:1032]).then_inc(nsems[0], 16)
            eng.dma_start(wt_t[:, :], wcat).then_inc(wsem, 16)
            eng.dma_start(xi[:, C + 516:C + 2568],
                          x[:, 4104:6156]).then_inc(nsems[1], 16)
            eng.dma_start(xi[:, C + 2568:C + 4620],
                          x[:, 6156:8208]).then_inc(nsems[2], 16)
            # thresholds: strip, then seg1 c0-3, then seg2 c0-1
            eng.wait_ge(psem, psem_idx(0, 0))
            eng.activation(out=e1[:, 4096:4608], in_=psum_dst(0, 0),
                           func=AF.Sign).then_inc(a3sem, 1)
            for k in range(4):
                eng.wait_ge(psem, psem_idx(1, k))
                eng.activation(out=e1[:, k * CHUNK:(k + 1) * CHUNK],
                               in_=psum_dst(1, k),
                               func=AF.Sign).then_inc(a1sem, 1)
            for k in (0, 1):
                eng.wait_ge(psem, psem_idx(2, k))
                eng.activation(out=e1[:, SEG_E[2] + k * CHUNK:
                                      SEG_E[2] + (k + 1) * CHUNK],
                               in_=psum_dst(2, k),
                               func=AF.Sign).then_inc(a2sem, 1)

    nc.compile()
    return nc


def make_in_maps(gtmasks):
    lab = np.asarray(gtmasks)[:, 0].astype(ml_dtypes.bfloat16)  # labels 0..19
    wcat = make_weights()
    # one extra bottom pad row so the strip's (unused) last DV row has data
    padded = [np.pad(lab[b], ((2, 3), (2, 2))) for b in range(B)]
    in_maps = []
    rows128 = np.arange(128)
    for c in range(NCORES):
        b, q = divmod(c, B * 2)  # 4 cores per batch
        xf = padded[b]
        base = RPC * q

        def seg_block(shift):
            s1 = xf[base + shift + rows128, :]                    # [128, 2052]
            s2 = xf[base + 124 + shift + rows128, :]              # [128, 2052]
            return np.concatenate([s1, s2], axis=1)

        def strip_block(shift):
            return xf[H - SR + shift: H - SR + shift + 36,
                      SC * q: SC * q + SEG_L[0]]

        xfull = np.concatenate([seg_block(0), seg_block(1)], axis=1)
        xstrip = np.concatenate([strip_block(0), strip_block(1)], axis=1)
        in_maps.append({"x": np.ascontiguousarray(xfull),
                        "xs": np.ascontiguousarray(xstrip), "wcat": wcat})
    return in_maps


def assemble(results):
    out = np.zeros((B, 1, H, W), np.int32)
    for c in range(NCORES):
        b, q = divmod(c, B * 2)
        y2 = results[c]["y2"]
        out[b, 0, RPC * q: RPC * q + 124, :] = y2[:, 0:2048]
        out[b, 0, RPC * q + 124: RPC * q + 248, :] = y2[:, 2048:4096]
        out[b, 0, H - SR:, SC * q: SC * q + SC] = y2[0:SR, 4096:4608]
    return out


def kernel(gtmasks):
    global LAST_EXEC_NS, LAST_RESULTS
    in_maps = make_in_maps(gtmasks)
    nc = build_nc()
    res = bass_utils.run_bass_kernel_spmd(
        nc, in_maps, core_ids=list(range(NCORES)), trace=PROFILE)
    LAST_EXEC_NS = res.exec_time_ns
    LAST_RESULTS = res
    return assemble(res.results)
